# revision 5
# baseline (speedup 1.0000x reference)
"""Trainium2 Bass kernel for nn_Decoding_model_23570780521049.

Normalized min-sum LDPC decoder: 63 checks x 127 vars, row weight 8,
batch 4096, 5 iterations.  Pure data parallelism: batch is sharded
across 8 NeuronCores (512 per core).

Per-core algorithm (see proto derivation):
  state curT (127 vars on partitions, 512 batch free), per iteration:
    - flipped gather matmul per 128-batch block:
        E[b, e] = curT[var(e), b]           (TensorE, G 0/1 matrix)
    - per-check reductions along free dim (b-part layout (128, 63, 8)):
        m1 = min |E|, eq = (|E| == m1), m2 = min(|E| + BIG*eq),
        S = sign(prod E)
    - check-space messages: A = norm*S*m1, Dd = norm*S*(m2 - m1)
    - per-edge correction cv2 = eq * Dd  (only argmin edge differs)
    - scatter: U[v,b] = H^T @ A  +  Sc^T @ cv2^T   (TensorE; cv2
      transposed to edge-partition layout via PE transposes)
    - update: cur += sign(cur) * U
Output: device writes the 5 iterates in var-part layout (5, 127, 512);
host transposes and stacks with the input snapshot.
"""

import numpy as np

M_CHECKS, N_CODE, RW = 63, 127, 8
E_TOT = M_CHECKS * RW          # 504
N_CORES = 8
B_CORE = 512                   # 4096 / 8
N_ITERS = 5
BIG = 1e10
_EW = [128, 128, 128, 120]     # edge-chunk widths (504 = 128*3 + 120)

_BUILD_CACHE = {}


def _make_H():
    rng = np.random.default_rng(0)
    H = np.zeros((M_CHECKS, N_CODE), dtype=np.int32)
    for i in range(M_CHECKS):
        H[i, rng.choice(N_CODE, RW, replace=False)] = 1
    return H


def _consts():
    H = _make_H()
    idx = np.stack([np.nonzero(H[i])[0] for i in range(M_CHECKS)])  # (63, 8)
    var_of_edge = idx.reshape(-1)
    G = np.zeros((N_CODE, E_TOT), np.float32)
    G[var_of_edge, np.arange(E_TOT)] = 1.0
    # scatter chunks, host layout (128 partitions, 4 chunks, 127)
    Sc = np.zeros((128, 4, N_CODE), np.float32)
    for e in range(E_TOT):
        Sc[e % 128, e // 128, var_of_edge[e]] = 1.0
    return H.astype(np.float32), G, Sc


def _build(norm: float):
    """Build + compile the per-core Bass module. Returns nc."""
    import concourse.bacc as bacc
    import concourse.mybir as mybir
    from concourse import masks
    from concourse.tile import TileContext

    f32 = mybir.dt.float32
    i32 = mybir.dt.int32
    AX = mybir.AxisListType
    OP = mybir.AluOpType
    AF = mybir.ActivationFunctionType

    nc = bacc.Bacc("TRN2", target_bir_lowering=False, debug=False)

    x_in = nc.dram_tensor("x0", [N_CODE, B_CORE], f32, kind="ExternalInput")
    g_in = nc.dram_tensor("gmat", [N_CODE, E_TOT], f32, kind="ExternalInput")
    sc_in = nc.dram_tensor("scmat", [128, 4, N_CODE], f32, kind="ExternalInput")
    h_in = nc.dram_tensor("hmat", [M_CHECKS, N_CODE], f32, kind="ExternalInput")
    y_out = nc.dram_tensor("y", [N_ITERS, N_CODE, B_CORE], f32,
                           kind="ExternalOutput")

    with TileContext(nc) as tc:
        with (
            tc.tile_pool(name="const", bufs=1) as cpool,
            tc.tile_pool(name="state", bufs=2) as spool,
            tc.tile_pool(name="work", bufs=3) as wpool,
            tc.tile_pool(name="small", bufs=4) as smpool,
            tc.tile_pool(name="asm", bufs=2) as apool,
            tc.tile_pool(name="eps", bufs=2, space="PSUM") as ps_e,
            tc.tile_pool(name="tps", bufs=3, space="PSUM") as ps_t,
            tc.tile_pool(name="ups", bufs=2, space="PSUM") as ps_u,
        ):
            g_sb = cpool.tile([N_CODE, E_TOT], f32)
            nc.sync.dma_start(g_sb[:], g_in[:])
            sc_sb = cpool.tile([128, 4, N_CODE], f32)
            nc.sync.dma_start(sc_sb[:], sc_in[:])
            h_sb = cpool.tile([M_CHECKS, N_CODE], f32)
            nc.sync.dma_start(h_sb[:], h_in[:])
            ident = cpool.tile([128, 128], f32)
            masks.make_identity(nc, ident[:])

            cur = spool.tile([N_CODE, B_CORE], f32, tag="cur")
            nc.sync.dma_start(cur[:], x_in[:])

            for it in range(N_ITERS):
                a_cp = apool.tile([M_CHECKS, B_CORE], f32, tag="acp")
                cvt = [apool.tile([_EW[c], B_CORE], f32, tag=f"cvt{c}",
                                  name=f"cvt{c}")
                       for c in range(4)]

                for g in range(4):
                    gs = slice(128 * g, 128 * (g + 1))
                    e_ps = ps_e.tile([128, E_TOT], f32, tag="eps")
                    nc.tensor.matmul(e_ps[:], cur[:, gs], g_sb[:],
                                     start=True, stop=True)
                    ev = e_ps[:].rearrange("p (c k) -> p c k", k=RW)

                    a = wpool.tile([128, E_TOT], f32, tag="a")
                    nc.scalar.activation(a[:], e_ps[:], AF.Abs)
                    av = a[:].rearrange("p (c k) -> p c k", k=RW)

                    m1 = smpool.tile([128, M_CHECKS], f32, tag="m1")
                    nc.vector.tensor_reduce(m1[:], av, axis=AX.X, op=OP.min)
                    m1b = m1[:].unsqueeze(2).broadcast_to((128, M_CHECKS, RW))

                    eq = wpool.tile([128, E_TOT], f32, tag="eq")
                    eqv = eq[:].rearrange("p (c k) -> p c k", k=RW)
                    nc.vector.tensor_tensor(eqv, av, m1b, op=OP.is_equal)

                    tb = wpool.tile([128, E_TOT], f32, tag="tb")
                    nc.vector.scalar_tensor_tensor(
                        tb[:], eq[:], BIG, a[:],
                        op0=OP.mult, op1=OP.add)
                    tbv = tb[:].rearrange("p (c k) -> p c k", k=RW)

                    m2 = smpool.tile([128, M_CHECKS], f32, tag="m2")
                    nc.vector.tensor_reduce(m2[:], tbv, axis=AX.X, op=OP.min)

                    # sign product via parity of negative count
                    nb = wpool.tile([128, E_TOT], f32, tag="nb")
                    nc.vector.tensor_scalar(nb[:], e_ps[:], 0.0, None,
                                            op0=OP.is_lt)
                    nbv = nb[:].rearrange("p (c k) -> p c k", k=RW)
                    cnt = smpool.tile([128, M_CHECKS], f32, tag="cnt")
                    nc.vector.tensor_reduce(cnt[:], nbv, axis=AX.X, op=OP.add)
                    pi = smpool.tile([128, M_CHECKS], i32, tag="pi")
                    nc.vector.tensor_copy(pi[:], cnt[:])
                    pb = smpool.tile([128, M_CHECKS], i32, tag="pb")
                    nc.vector.tensor_scalar(pb[:], pi[:], 1, None,
                                            op0=OP.bitwise_and)
                    S = smpool.tile([128, M_CHECKS], f32, tag="S")
                    nc.vector.tensor_scalar(S[:], pb[:], -2, 1,
                                            op0=OP.mult, op1=OP.add)

                    d = smpool.tile([128, M_CHECKS], f32, tag="d")
                    nc.vector.tensor_tensor(d[:], m2[:], m1[:], op=OP.subtract)
                    A = smpool.tile([128, M_CHECKS], f32, tag="A")
                    nc.vector.scalar_tensor_tensor(
                        A[:], S[:], float(norm), m1[:],
                        op0=OP.mult, op1=OP.mult)
                    Dd = smpool.tile([128, M_CHECKS], f32, tag="Dd")
                    nc.vector.scalar_tensor_tensor(
                        Dd[:], S[:], float(norm), d[:],
                        op0=OP.mult, op1=OP.mult)
                    Ddb = Dd[:].unsqueeze(2).broadcast_to((128, M_CHECKS, RW))

                    cv2 = wpool.tile([128, E_TOT], f32, tag="cv2")
                    cv2v = cv2[:].rearrange("p (c k) -> p c k", k=RW)
                    nc.vector.tensor_tensor(cv2v, eqv, Ddb, op=OP.mult)

                    # transpose A (128, 63) -> (63, 128) and place into a_cp
                    at_ps = ps_t.tile([M_CHECKS, 128], f32, tag="tp", name="at_ps")
                    nc.tensor.transpose(at_ps[:], A[:], ident[:])
                    nc.scalar.copy(a_cp[:, gs], at_ps[:])

                    # transpose cv2 chunks -> edge-partition layout
                    for c in range(4):
                        w = _EW[c]
                        ct_ps = ps_t.tile([128, 128], f32, tag="tp", name="ct_ps")
                        nc.tensor.transpose(
                            ct_ps[:w, :], cv2[:, 128 * c:128 * c + w],
                            ident[:])
                        if c % 2 == 0:
                            nc.scalar.copy(cvt[c][:, gs], ct_ps[:w, :])
                        else:
                            nc.vector.tensor_copy(cvt[c][:, gs], ct_ps[:w, :])

                # scatter: U = H^T @ A_cp + sum_c Sc_c^T @ cvt_c
                u_ps = ps_u.tile([N_CODE, B_CORE], f32, tag="ups")
                nc.tensor.matmul(u_ps[:], h_sb[:], a_cp[:],
                                 start=True, stop=False)
                for c in range(4):
                    w = _EW[c]
                    nc.tensor.matmul(u_ps[:], sc_sb[:w, c, :], cvt[c][:],
                                     start=False, stop=(c == 3))

                sg = wpool.tile([N_CODE, B_CORE], f32, tag="sg")
                nc.scalar.sign(sg[:], cur[:])
                u2 = wpool.tile([N_CODE, B_CORE], f32, tag="u2")
                nc.vector.tensor_tensor(u2[:], sg[:], u_ps[:], op=OP.mult)
                newcur = spool.tile([N_CODE, B_CORE], f32, tag="cur")
                nc.vector.tensor_tensor(newcur[:], cur[:], u2[:], op=OP.add)
                nc.sync.dma_start(y_out[it], newcur[:])
                cur = newcur

    nc.compile()
    return nc


def _get_nc(norm: float):
    key = round(float(norm), 9)
    if key not in _BUILD_CACHE:
        _BUILD_CACHE[key] = _build(float(norm))
    return _BUILD_CACHE[key]


def kernel(soft_input, labels, H, normalizor):
    from concourse.bass_utils import run_bass_kernel_spmd

    soft_input = np.asarray(soft_input, dtype=np.float32)
    labels = np.asarray(labels)
    norm = float(np.log1p(np.exp(np.float32(np.asarray(normalizor).ravel()[0]))))

    nc = _get_nc(norm)
    Hf, G, Sc = _consts()

    in_maps = []
    for c in range(N_CORES):
        sl = soft_input[c * B_CORE:(c + 1) * B_CORE]          # (512, 127)
        in_maps.append({
            "x0": np.ascontiguousarray(sl.T),                  # (127, 512)
            "gmat": G,
            "scmat": Sc,
            "hmat": Hf,
        })

    res = run_bass_kernel_spmd(nc, in_maps, core_ids=list(range(N_CORES)))
    outs = []
    for c in range(N_CORES):
        y = res.results[c]["y"]                                # (5, 127, 512)
        outs.append(np.transpose(y, (0, 2, 1)))                # (5, 512, 127)
    dev = np.concatenate(outs, axis=1)                         # (5, 4096, 127)
    full = np.concatenate([soft_input[None], dev], axis=0)     # (6, 4096, 127)
    return full, labels


# revision 11
# speedup vs baseline: 1.2598x; 1.2598x over previous
"""Trainium2 Bass kernel for nn_Decoding_model_23570780521049.

Normalized min-sum LDPC decoder: 63 checks x 127 vars, row weight 8,
batch 4096, 5 iterations.  Pure data parallelism: batch is sharded
across 8 NeuronCores (512 per core).

Per-core algorithm:
  state curT (127 vars on partitions, 512 batch free), per iteration:
    - flipped gather matmul per 128-batch block (TensorE, 0/1 matrix G):
        E[b, e] = curT[var(e), b]   -> PSUM, batch-partition layout
    - per-check reductions along the free dim on (128, 63, 8) views:
        m1 = min |E|  (reduce with abs)
        t  = |E| + BIG*(|E| == m1)      (custom DVE op)
        m2 = min t
        negative-count via flipped matmul with H^T, parity -> S = +-1
    - check-space messages: A = norm*S*m1, Dd = norm*S*(m2 - m1)
    - per-edge correction cv2 = (t >= BIG/2) ? Dd : 0  (custom DVE op)
    - scatter (TensorE): U = H^T @ A_cp + sum_c Sc_c^T @ cv2T_c, where
      cv2 is moved to edge-partition layout via PE transposes
    - update (custom DVE op): cur += sign(cur) * U
Output: device writes the 5 iterates in var-part layout (5, 127, 512);
host transposes and stacks with the input snapshot.
"""

import numpy as np

M_CHECKS, N_CODE, RW = 63, 127, 8
E_TOT = M_CHECKS * RW          # 504
N_CORES = 8
B_CORE = 512                   # 4096 / 8
N_ITERS = 5
BIG = 1e10
_EW = [128, 128, 128, 120]     # edge-chunk widths (504 = 128*3 + 120)

_BUILD_CACHE = {}
_OPS_CACHE = {}


def _make_H():
    rng = np.random.default_rng(0)
    H = np.zeros((M_CHECKS, N_CODE), dtype=np.int32)
    for i in range(M_CHECKS):
        H[i, rng.choice(N_CODE, RW, replace=False)] = 1
    return H


def _consts():
    H = _make_H()
    idx = np.stack([np.nonzero(H[i])[0] for i in range(M_CHECKS)])  # (63, 8)
    var_of_edge = idx.reshape(-1)
    G = np.zeros((N_CODE, E_TOT), np.float32)
    G[var_of_edge, np.arange(E_TOT)] = 1.0
    # scatter chunks, host layout (128 partitions, 4 chunks, 127)
    Sc = np.zeros((128, 4, N_CODE), np.float32)
    for e in range(E_TOT):
        Sc[e % 128, e // 128, var_of_edge[e]] = 1.0
    return H.astype(np.float32), G, Sc


def _register_ops():
    """Register the fused DVE ops via the documented dve_ops extension API."""
    if _OPS_CACHE:
        return _OPS_CACHE

    import concourse.dve_ops as dve_ops
    from concourse.dve_ops import DveOp
    from concourse.dve_spec import (
        Spec, Src0, Src1, Zero, One, C0, C1, C2, maxx, eq, select, lower,
        _has_src1,
    )
    from concourse.dve_uop import DveOpSpec

    def _mk(name, spec):
        if name in dve_ops._SUB_OPCODE_FOR_NAME:
            return next(op for op in dve_ops.OPS if op.name == name)
        shas = {}
        for ver in ("v3", "v4"):
            s = DveOpSpec(name=name, opcode=0, uops=lower(spec, ver=ver),
                          rd1_en=_has_src1(spec))
            shas[ver] = s.sha(ver)
        op = DveOp(name, spec, subdim=False, uops_sha=shas)
        dve_ops.OPS.append(op)
        dve_ops.CUSTOM_DVE_SPECS[name] = spec
        dve_ops._SUB_OPCODE_FOR_NAME[name] = (
            dve_ops._CUSTOM_DVE_ROW_BASE + len(dve_ops.OPS) - 1)
        assert dve_ops._SUB_OPCODE_FOR_NAME[name] < 0x20
        return op

    _am = maxx(Src0, Zero - Src0)
    _OPS_CACHE["teq"] = _mk(
        "LDPC_TEQ",
        Spec(
            body=_am + eq(_am, Src1) * C0,
            reference=lambda in0, in1, s0, s1, imm2:
                (lambda am: am + (am == np.reshape(in1, am.shape))
                 * np.float32(s0))(np.abs(in0)),
        ),
    )
    _OPS_CACHE["selge"] = _mk(
        "LDPC_SELGE",
        Spec(
            body=select(Src0 >= C0, Src1, Zero),
            reference=lambda in0, in1, s0, s1, imm2:
                np.where(in0 >= np.float32(s0), np.reshape(in1, in0.shape),
                         0.0).astype(np.float32),
        ),
    )
    # S = 1 - 4*|cnt/2 - round(cnt/2)|  (exact +-1 parity for small ints;
    # round via the 2^23 add/sub trick, ties irrelevant: |frac| is 0 or 0.5)
    _z = Src0 * C0
    _w = _z + C1
    _r = _w - C1
    _f = _z - _r
    _af = maxx(_f, Zero - _f)
    _OPS_CACHE["parity"] = _mk(
        "LDPC_PARITY",
        Spec(
            body=One - _af * C2,
            reference=lambda in0, in1, s0, s1, imm2:
                (1.0 - imm2 * np.abs(in0 * np.float32(s0)
                 - np.round(in0 * np.float32(s0)))).astype(np.float32),
        ),
    )
    _OPS_CACHE["signadd"] = _mk(
        "LDPC_SIGNADD",
        Spec(
            body=Src0 + select(Src0 < Zero, Zero - Src1, Src1),
            reference=lambda in0, in1, s0, s1, imm2:
                (in0 + np.where(in0 < 0, -in1, in1)).astype(np.float32),
        ),
    )
    return _OPS_CACHE


def _build():
    """Build + compile the per-core Bass module. Returns nc."""
    import concourse.bacc as bacc
    import concourse.mybir as mybir
    from concourse import masks
    from concourse.tile import TileContext

    ops = _register_ops()

    f32 = mybir.dt.float32
    i32 = mybir.dt.int32
    AX = mybir.AxisListType
    OP = mybir.AluOpType
    AF = mybir.ActivationFunctionType

    nc = bacc.Bacc("TRN2", target_bir_lowering=False, debug=False)

    x_in = nc.dram_tensor("x0", [N_CODE, B_CORE], f32, kind="ExternalInput")
    g_in = nc.dram_tensor("gmat", [N_CODE, E_TOT], f32, kind="ExternalInput")
    sc_in = nc.dram_tensor("scmat", [128, 4, N_CODE], f32, kind="ExternalInput")
    h_in = nc.dram_tensor("hmat", [M_CHECKS, N_CODE], f32, kind="ExternalInput")
    ht_in = nc.dram_tensor("htmat", [N_CODE, M_CHECKS], f32, kind="ExternalInput")
    y_out = nc.dram_tensor("y", [N_ITERS, N_CODE, B_CORE], f32,
                           kind="ExternalOutput")

    with TileContext(nc) as tc:
        with (
            tc.tile_pool(name="const", bufs=1) as cpool,
            tc.tile_pool(name="state", bufs=2) as spool,
            tc.tile_pool(name="work", bufs=3) as wpool,
            tc.tile_pool(name="small", bufs=4) as smpool,
            tc.tile_pool(name="asm", bufs=2) as apool,
            tc.tile_pool(name="eps", bufs=2, space="PSUM") as ps_e,
            tc.tile_pool(name="tps", bufs=3, space="PSUM") as ps_t,
            tc.tile_pool(name="ups", bufs=2, space="PSUM") as ps_u,
        ):
            g_sb = cpool.tile([N_CODE, E_TOT], f32)
            nc.sync.dma_start(g_sb[:], g_in[:])
            sc_sb = cpool.tile([128, 4, N_CODE], f32)
            nc.sync.dma_start(sc_sb[:], sc_in[:])
            h_sb = cpool.tile([M_CHECKS, N_CODE], f32)
            nc.sync.dma_start(h_sb[:], h_in[:])
            ht_sb = cpool.tile([N_CODE, M_CHECKS], f32)
            nc.sync.dma_start(ht_sb[:], ht_in[:])
            ident = cpool.tile([128, 128], f32)
            masks.make_identity(nc, ident[:])

            cur = spool.tile([N_CODE, B_CORE], f32, tag="cur")
            nc.sync.dma_start(cur[:], x_in[:])

            for it in range(N_ITERS):
                a_cp = apool.tile([M_CHECKS, B_CORE], f32, tag="acp")
                cvt = [apool.tile([_EW[c], B_CORE], f32, tag=f"cvt{c}",
                                  name=f"cvt{c}")
                       for c in range(4)]

                # negative indicator over the whole state (2x-mode DVE op)
                nbv = wpool.tile([N_CODE, B_CORE], f32, tag="nbv")
                nc.vector.tensor_scalar(nbv[:], cur[:], 0.0, None,
                                        op0=OP.is_lt)

                for g in range(4):
                    gs = slice(128 * g, 128 * (g + 1))
                    e_ps = ps_e.tile([128, E_TOT], f32, tag="eps")
                    nc.tensor.matmul(e_ps[:], cur[:, gs], g_sb[:],
                                     start=True, stop=True)
                    ev = e_ps[:].rearrange("p (c k) -> p c k", k=RW)

                    m1 = smpool.tile([128, M_CHECKS], f32, tag="m1")
                    nc.vector.tensor_reduce(m1[:], ev, axis=AX.X, op=OP.min,
                                            apply_absolute_value=True)
                    m1b = m1[:].unsqueeze(2).broadcast_to((128, M_CHECKS, RW))

                    # t = |E| + BIG*(|E| == m1)   (fused custom op)
                    tb = wpool.tile([128, E_TOT], f32, tag="tb")
                    tbv = tb[:].rearrange("p (c k) -> p c k", k=RW)
                    nc.vector._custom_dve(ops["teq"], out=tbv, in0=ev,
                                          in1=m1b, s0=BIG)

                    m2 = smpool.tile([128, M_CHECKS], f32, tag="m2")
                    nc.vector.tensor_reduce(m2[:], tbv, axis=AX.X, op=OP.min)

                    # negative count via flipped matmul, then parity -> S
                    cnt_ps = ps_t.tile([128, M_CHECKS], f32, tag="tp",
                                       name="cnt_ps")
                    nc.tensor.matmul(cnt_ps[:], nbv[:, gs], ht_sb[:],
                                     start=True, stop=True)
                    # S = +-1 from parity of cnt (custom DVE op)
                    S = smpool.tile([128, M_CHECKS], f32, tag="S")
                    nc.vector._custom_dve(ops["parity"], out=S[:],
                                          in0=cnt_ps[:], s0=0.5,
                                          s1=8388608.0, imm2=4.0)

                    d = smpool.tile([128, M_CHECKS], f32, tag="d")
                    nc.gpsimd.tensor_tensor(d[:], m2[:], m1[:],
                                            op=OP.subtract)
                    A = smpool.tile([128, M_CHECKS], f32, tag="A")
                    nc.gpsimd.tensor_tensor(A[:], S[:], m1[:], op=OP.mult)
                    Dd = smpool.tile([128, M_CHECKS], f32, tag="Dd")
                    nc.gpsimd.tensor_tensor(Dd[:], S[:], d[:], op=OP.mult)
                    Ddb = Dd[:].unsqueeze(2).broadcast_to((128, M_CHECKS, RW))

                    # cv2 = (t >= BIG/2) ? Dd : 0   (fused custom op)
                    cv2 = wpool.tile([128, E_TOT], f32, tag="cv2")
                    cv2v = cv2[:].rearrange("p (c k) -> p c k", k=RW)
                    nc.vector._custom_dve(ops["selge"], out=cv2v, in0=tbv,
                                          in1=Ddb, s0=BIG * 0.5)

                    # transpose A (128, 63) -> (63, 128) and place into a_cp
                    at_ps = ps_t.tile([M_CHECKS, 128], f32, tag="tp",
                                      name="at_ps")
                    nc.tensor.transpose(at_ps[:], A[:], ident[:])
                    nc.scalar.copy(a_cp[:, gs], at_ps[:])

                    # transpose cv2 chunks -> edge-partition layout
                    for c in range(4):
                        w = _EW[c]
                        ct_ps = ps_t.tile([128, 128], f32, tag="tp",
                                          name="ct_ps")
                        nc.tensor.transpose(
                            ct_ps[:w, :], cv2[:, 128 * c:128 * c + w],
                            ident[:])
                        nc.scalar.copy(cvt[c][:, gs], ct_ps[:w, :])

                # scatter: U = H^T @ A_cp + sum_c Sc_c^T @ cvt_c
                u_ps = ps_u.tile([N_CODE, B_CORE], f32, tag="ups")
                nc.tensor.matmul(u_ps[:], h_sb[:], a_cp[:],
                                 start=True, stop=False)
                for c in range(4):
                    w = _EW[c]
                    nc.tensor.matmul(u_ps[:], sc_sb[:w, c, :], cvt[c][:],
                                     start=False, stop=(c == 3))

                # cur += sign(cur) * U   (fused custom op)
                newcur = spool.tile([N_CODE, B_CORE], f32, tag="cur")
                nc.vector._custom_dve(ops["signadd"], out=newcur[:],
                                      in0=cur[:], in1=u_ps[:])
                nc.sync.dma_start(y_out[it], newcur[:])
                cur = newcur

    nc.compile()
    return nc


def _get_nc():
    if "nc" not in _BUILD_CACHE:
        _BUILD_CACHE["nc"] = _build()
    return _BUILD_CACHE["nc"]


def kernel(soft_input, labels, H, normalizor):
    from concourse.bass_utils import run_bass_kernel_spmd

    soft_input = np.asarray(soft_input, dtype=np.float32)
    labels = np.asarray(labels)
    norm = float(np.log1p(np.exp(np.float32(np.asarray(normalizor).ravel()[0]))))

    nc = _get_nc()
    Hf, G, Sc = _consts()

    in_maps = []
    for c in range(N_CORES):
        sl = soft_input[c * B_CORE:(c + 1) * B_CORE]          # (512, 127)
        in_maps.append({
            "x0": np.ascontiguousarray(sl.T),                  # (127, 512)
            "gmat": G,
            "scmat": Sc * np.float32(norm),
            "hmat": Hf * np.float32(norm),
            "htmat": np.ascontiguousarray(Hf.T),
        })

    res = run_bass_kernel_spmd(nc, in_maps, core_ids=list(range(N_CORES)))
    outs = []
    for c in range(N_CORES):
        y = res.results[c]["y"]                                # (5, 127, 512)
        outs.append(np.transpose(y, (0, 2, 1)))                # (5, 512, 127)
    dev = np.concatenate(outs, axis=1)                         # (5, 4096, 127)
    full = np.concatenate([soft_input[None], dev], axis=0)     # (6, 4096, 127)
    return full, labels
